# revision 16
# baseline (speedup 1.0000x reference)
"""BitLinear (LayerNorm + 8-bit act quant + ternary weight quant + GEMM) on 8 TRN2 cores.

Sharding: data-parallel over flattened rows (B*S = 8192 -> 1024 rows/core).
Each core holds the full quantized transposed weight (bf16, exact for ternary
values) and computes LN + activation quantization for its own rows only, so
there is no redundant vector work and no collective.

Weight quantization (absmean ternary) is precomputed on the host: the weight
scale gamma is a single global scalar and the quantized weights are static --
the standard BitLinear inference setup (the sharding hint explicitly allows
precomputing the weight scale). The weight is pre-tiled on the host so every
device DMA is a single fully contiguous stream.

v2 structure (per core, per rep):
  phase A (per 128-row tile): DMA x -> ACT computes row sum/sumsq via
  accum_out while DVE computes row max/min; tiny [P,1] chain produces
  mu, rstd, eta and the fused affine C,D; one DVE pass computes
  round-ready values in place; ACT casts to bf16; ONE blocked xbar DMA
  transpose produces x_q^T in [k_inner, k_tile, m_inner] layout.
  phase B: stream w^T chunks, 16-matmul PSUM accumulation groups
  (bf16, exact for the integer values), ACT applies the per-row output
  scale while evacuating PSUM -> SBUF, DMA out.

Numerics: x_q in [-127,127] and w_q in {-1,0,1} are exact in bf16; products
(<=127) and fp32 PSUM accumulation (sums < 2^19) are exact. Rounding uses the
+/-1.5*2^23 trick (round-to-nearest-even, like jnp.round). mean/var come from
ACT-accumulated sum/sumsq (var = E[x^2]-mu^2; no cancellation risk since
mu ~ 0, var ~ 1 for LN inputs); absmax((x-mu)*rstd) = rstd*max(max-mu, mu-min).
"""

import numpy as np
import ml_dtypes

import concourse.bass as bass
import concourse.bacc as bacc
import concourse.mybir as mybir
import concourse.tile as tile
from concourse.bass_utils import run_bass_kernel_spmd

# Problem shapes (hardcoded per contract -- kernel.py must be self-contained).
B, S, K, N = 2, 4096, 2048, 8192
M_TOTAL = B * S              # 8192 flattened rows
N_CORES = 8
M_LOC = M_TOTAL // N_CORES   # 1024 rows per core
P = 128                      # partitions
M_TILES = M_LOC // P         # 8
K_TILES = K // P             # 16
N_MM = 512                   # moving-operand free dim per matmul (1 PSUM bank)
N_CHUNK = 512                # weight-stream / output-store chunk along N
N_CHUNKS = N // N_CHUNK      # 16

EPS_LN = 1e-5
EPS_Q = 1e-5
MAGIC = 12582912.0           # 1.5 * 2**23: fp32 add/sub performs round-to-nearest-even

FP32 = mybir.dt.float32
BF16 = mybir.dt.bfloat16
ACT_COPY = mybir.ActivationFunctionType.Copy
ACT_SQUARE = mybir.ActivationFunctionType.Square
ACT_SQRT = mybir.ActivationFunctionType.Sqrt
ALU = mybir.AluOpType
AX_X = mybir.AxisListType.X


def _build_nc(reps=1, with_bias=False):
    nc = bacc.Bacc("TRN2", target_bir_lowering=False, debug=False, num_devices=N_CORES)

    xc_d = nc.dram_tensor("xc", [M_LOC, K], FP32, kind="ExternalInput")
    # host-pretiled weight: [nch, p, kt, n_in_chunk] so each chunk DMA is contiguous
    wt_d = nc.dram_tensor("wt", [N_CHUNKS, P, K_TILES, N_CHUNK], BF16, kind="ExternalInput")
    gs_d = nc.dram_tensor("gs", [P, 1], FP32, kind="ExternalInput")   # gamma/127 replicated
    if with_bias:
        bs_d = nc.dram_tensor("bs", [N], FP32, kind="ExternalInput")
    out_d = nc.dram_tensor("out", [M_LOC, N], FP32, kind="ExternalOutput")

    with tile.TileContext(nc) as tc:
        with (
            tc.tile_pool(name="singles", bufs=1) as singles,
            tc.tile_pool(name="xin", bufs=3) as xin_pool,
            tc.tile_pool(name="xq", bufs=2) as xq_pool,
            tc.tile_pool(name="sq", bufs=2) as sq_pool,
            tc.tile_pool(name="stats", bufs=4) as stats_pool,
            tc.tile_pool(name="xqt", bufs=2) as xqt_pool,
            tc.tile_pool(name="rs", bufs=2) as rs_pool,
            tc.tile_pool(name="wstream", bufs=3) as w_pool,
            tc.tile_pool(name="w0", bufs=1) as w0_pool,
            tc.tile_pool(name="osb", bufs=4) as o_pool,
            tc.tile_pool(name="psg", bufs=8, space="PSUM") as psg_pool,
        ):
            # --- constants ---
            gs_t = singles.tile([P, 1], FP32)
            nc.sync.dma_start(gs_t[:], gs_d[:])
            eps_t = singles.tile([P, 1], FP32)
            nc.vector.memset(eps_t[:], EPS_LN)
            zero_t = singles.tile([P, 1], FP32)
            nc.vector.memset(zero_t[:], 0.0)
            trash = singles.tile([P, K], BF16)   # dead store for accum-only ACT pass
            if with_bias:
                bias_t = singles.tile([P, N], FP32)
                bias_bcast = bass.AP(
                    tensor=bs_d.ap().tensor, offset=0, ap=[[0, P]] + bs_d.ap().ap
                )
                nc.sync.dma_start(bias_t[:], bias_bcast)

            for _rep in range(reps):
                # x_q^T, bf16, laid out [p=k_inner, m_tile, k_tile, m_inner]
                xqt = xqt_pool.tile([P, M_TILES, K_TILES, P], BF16)
                rs_all = rs_pool.tile([P, M_TILES], FP32)  # per-row output scale
                wtile0 = None

                # --- phase A: LayerNorm + activation quant + transpose ---
                for m in range(M_TILES):
                    xt = xin_pool.tile([P, K], FP32)
                    if m < 2 and _rep == 0:
                        # bootstrap: only the first tiles' loads jump the queue
                        with tc.high_priority():
                            nc.sync.dma_start(xt[:], xc_d[m * P : (m + 1) * P, :])
                    else:
                        nc.sync.dma_start(xt[:], xc_d[m * P : (m + 1) * P, :])
                    if m == 0:
                        # prefetch weight chunks 0+1 early so phase B can start
                        # as soon as the first x tile is quantized
                        wtile0 = w0_pool.tile([P, 2, K_TILES, N_CHUNK], BF16)
                        nc.sync.dma_start(
                            wtile0[:, 0], wt_d[0]
                        )
                        nc.sync.dma_start(
                            wtile0[:, 1], wt_d[1]
                        )

                    st = stats_pool.tile([P, 8], FP32)
                    sums = st[:, 0:1]
                    ssq = st[:, 1:2]
                    # ACT pass 1: row sum (out tile is a dead store)
                    nc.scalar.activation(trash[:], xt[:], ACT_COPY, accum_out=sums)
                    mu = st[:, 2:3]
                    nc.vector.tensor_scalar_mul(out=mu, in0=sums, scalar1=1.0 / K)
                    nmu = st[:, 3:4]
                    nc.vector.tensor_scalar_mul(out=nmu, in0=mu, scalar1=-1.0)
                    # ACT pass 2: sq = (x-mu)^2 (fp32), accum -> sum((x-mu)^2)
                    sq = sq_pool.tile([P, K], FP32)
                    nc.scalar.activation(
                        sq[:], xt[:], ACT_SQUARE, bias=nmu, accum_out=ssq
                    )
                    # DVE: max((x-mu)^2)  ->  absmax = sqrt
                    msq = st[:, 4:5]
                    nc.vector.tensor_reduce(out=msq, in_=sq[:], axis=AX_X, op=ALU.max)
                    # std = sqrt(ssq/K + eps); absm = sqrt(msq)
                    std = st[:, 5:6]
                    nc.scalar.activation(std, ssq, ACT_SQRT, bias=eps_t[:], scale=1.0 / K)
                    absm = st[:, 6:7]
                    nc.scalar.activation(absm, msq, ACT_SQRT, bias=zero_t[:], scale=1.0)
                    rstd = st[:, 7:8]
                    nc.vector.reciprocal(rstd, std)

                    # eta = max(absm*rstd, EPS_Q)
                    eta = st[:, 0:1]      # reuse sums slot
                    nc.vector.tensor_scalar(
                        out=eta, in0=absm, scalar1=rstd, scalar2=EPS_Q,
                        op0=ALU.mult, op1=ALU.max,
                    )
                    nc.vector.tensor_mul(rs_all[:, m : m + 1], eta, gs_t[:])
                    inv = st[:, 1:2]      # reuse ssq slot
                    nc.vector.reciprocal(inv, eta)
                    # C = rstd * 127 * inv
                    cf = st[:, 4:5]       # reuse msq slot
                    nc.vector.tensor_scalar(
                        out=cf, in0=inv, scalar1=127.0, scalar2=rstd,
                        op0=ALU.mult, op1=ALU.mult,
                    )
                    # quant: xt <- (x - mu)*C, then +MAGIC as a separate fp32
                    # add (RNE at integer; MAGIC must NOT be folded into a
                    # combined constant or the mu offset quantizes to ulp(MAGIC))
                    nc.vector.tensor_scalar(
                        out=xt[:], in0=xt[:], scalar1=mu, scalar2=cf,
                        op0=ALU.subtract, op1=ALU.mult,
                    )
                    nc.vector.tensor_scalar_add(out=xt[:], in0=xt[:], scalar1=MAGIC)
                    # subtract MAGIC, cast to bf16 (ints <= 127: exact);
                    # split across ACT and Pool so neither engine bottlenecks
                    # split cast across ACT and Pool to cut latency
                    xq = xq_pool.tile([P, K], BF16)
                    nc.scalar.activation(
                        xq[:, : K // 2], xt[:, : K // 2], ACT_COPY, bias=-MAGIC
                    )
                    nc.gpsimd.tensor_scalar_add(
                        out=xq[:, K // 2 :], in0=xt[:, K // 2 :], scalar1=-MAGIC
                    )

                    # ONE blocked xbar transpose: xqt[p, m, kt, c] = xq[c, kt*P+p]
                    # (high priority: it gates the PE directly; output stores don't)
                    with tc.high_priority():
                        nc.sync.dma_start_transpose(xqt[:, m], xq[:])

                # --- phase B: GEMM + epilogue ---
                # Chunks 0+1 run m-major-paired so early consumption of xqt
                # tiles matches phase A's production rate; later chunks run
                # sequentially (xqt fully resident by then).
                def _gemm_group(nch, m, wt_ap):
                    pt = psg_pool.tile([P, N_MM], FP32)
                    for kt in range(K_TILES):
                        nc.tensor.matmul(
                            pt[:],
                            xqt[:, m, kt, :],
                            wt_ap(kt),
                            start=(kt == 0),
                            stop=(kt == K_TILES - 1),
                        )
                    osb = o_pool.tile([P, N_CHUNK], FP32)
                    # scale by per-row rs while evacuating PSUM
                    nc.vector.tensor_scalar_mul(
                        out=osb[:], in0=pt[:], scalar1=rs_all[:, m : m + 1],
                    )
                    if with_bias:
                        n0 = nch * N_CHUNK
                        nc.gpsimd.tensor_add(
                            osb[:], osb[:], bias_t[:, n0 : n0 + N_CHUNK]
                        )
                    nc.sync.dma_start(
                        out_d[m * P : (m + 1) * P,
                              nch * N_CHUNK : (nch + 1) * N_CHUNK],
                        osb[:],
                    )

                for m in range(M_TILES):
                    _gemm_group(0, m, lambda kt: wtile0[:, 0, kt, :])
                    _gemm_group(1, m, lambda kt: wtile0[:, 1, kt, :])
                wt23 = []
                for c in (2, 3):
                    wtile = w_pool.tile([P, K_TILES, N_CHUNK], BF16)
                    nc.sync.dma_start(wtile[:], wt_d[c])
                    wt23.append(wtile)
                for m in range(M_TILES):
                    for c in (2, 3):
                        _gemm_group(c, m, lambda kt, _w=wt23[c - 2]: _w[:, kt, :])
                for nch in range(4, N_CHUNKS):
                    wtile = w_pool.tile([P, K_TILES, N_CHUNK], BF16)
                    nc.sync.dma_start(wtile[:], wt_d[nch])
                    for m in range(M_TILES):
                        _gemm_group(nch, m, lambda kt: wtile[:, kt, :])

    nc.compile()
    return nc


_NC_CACHE = {}


def _get_nc(with_bias):
    key = with_bias
    if key not in _NC_CACHE:
        _NC_CACHE[key] = _build_nc(with_bias=with_bias)
    return _NC_CACHE[key]


def _weight_gamma(weight: np.ndarray) -> np.float32:
    """absmean scale, matching jnp.maximum(jnp.mean(jnp.abs(w)), EPS_Q) bitwise
    where possible (jax-cpu), falling back to float64 numpy."""
    try:
        import jax
        import jax.numpy as jnp

        w_cpu = jax.device_put(np.asarray(weight), jax.devices("cpu")[0])
        g = jnp.maximum(jnp.mean(jnp.abs(w_cpu)), EPS_Q)
        return np.float32(np.asarray(g))
    except Exception:
        return np.float32(max(np.mean(np.abs(weight), dtype=np.float64), EPS_Q))


def _prep_weight(weight: np.ndarray):
    gamma = _weight_gamma(weight)
    w_q = np.round(np.clip(weight.astype(np.float32) / gamma, -1.0, 1.0))
    # [N, K] -> wT [K, N] -> tiles [nch, p, kt, n_in_chunk], contiguous per chunk
    wt = np.ascontiguousarray(w_q.T)                       # [K, N]
    wt = wt.reshape(K_TILES, P, N_CHUNKS, N_CHUNK)         # [kt, p, nch, n]
    wt = np.ascontiguousarray(wt.transpose(2, 1, 0, 3))    # [nch, p, kt, n]
    return gamma, wt.astype(ml_dtypes.bfloat16)


def kernel(x: np.ndarray, weight: np.ndarray, bias: np.ndarray) -> np.ndarray:
    assert x.shape == (B, S, K) and weight.shape == (N, K) and bias.shape == (N,)

    gamma, wt_bf16 = _prep_weight(weight)
    gs = np.full((P, 1), gamma / np.float32(127.0), dtype=np.float32)
    bias_f = np.ascontiguousarray(bias.astype(np.float32))
    with_bias = bool(np.any(bias_f != 0.0))
    x_flat = np.ascontiguousarray(x.reshape(M_TOTAL, K).astype(np.float32))

    nc = _get_nc(with_bias)
    in_maps = []
    for c in range(N_CORES):
        m = {
            "xc": x_flat[c * M_LOC : (c + 1) * M_LOC],
            "wt": wt_bf16,
            "gs": gs,
        }
        if with_bias:
            m["bs"] = bias_f
        in_maps.append(m)
    res = run_bass_kernel_spmd(nc, in_maps, list(range(N_CORES)))
    out = np.concatenate([res.results[c]["out"] for c in range(N_CORES)], axis=0)
    return out.reshape(B, S, N).astype(np.float32, copy=False)
